# revision 26
# baseline (speedup 1.0000x reference)
"""MQA (GQA with 1 KV group) attention kernel for 8 Trainium2 NeuronCores.

Sharding: core c -> batch b = c//4, head-group hg = c%4 (4 of 16 query heads).
Each core computes Q/K/V projections from x[b]^T, causal attention for its 4
heads in transposed layout (S^T[kv, q] tiles), and a partial output
projection out_partial = A_h @ Wo[:, cols_h]^T.  Host sums the 4 bf16
partials per batch and adds bo.

Software-pipelined structure (everything bf16, f32 PSUM):
- The s-chunk projections are emitted interleaved with the attention q-chunk
  loop: attention instructions carry higher scheduler priority, projection
  matmuls fill tensor-engine gaps while the scalar engine runs exp, keeping
  the PE warm (HAM) and both engines busy.
- Attention per (q-chunk, head): S^T = K_tile^T Q, exp on the scalar engine
  (padding bias fused), multiplicative triangular mask on the [128,128]
  diagonal block only, A V accumulation, and a ones-matrix matmul that
  yields softmax row-sums pre-broadcast across all 128 partitions.
  Normalization via reciprocal_approx_fast.  Fully-masked columns of
  diagonal k-tiles are never computed (column ranges trimmed).
- All PSUM evacuations run on the vector engine (tensor_scalar_add for the
  biased projections) so the scalar engine does nothing but exp.
- PSUM budget: psS 2 + psO 2 + psR 2 + proj-chain 1 + transpose 1 = 8.
- Inputs host-packed partition-contiguous; output partials bf16.
"""

import sys

sys.path.insert(0, "/opt/trn_rl_repo")

import ml_dtypes
import numpy as np

import concourse.bass as bass
import concourse.tile as tile
from concourse import bacc
from concourse import mybir
from concourse.bass import ts
from concourse.bass_utils import run_bass_kernel_spmd
from concourse.masks import make_identity

B, S, HID = 2, 2048, 2048
H, D = 16, 128
HPC = 4              # heads per core
DPH = HPC * D        # 512: head dims per core
NCORES = 8
SC = 512             # s-chunk (free dim for most matmuls)
NSC = S // SC        # 4
NT = S // 128        # 16 128-tiles along s / hid
NHT = HID // 128     # 16 hid tiles
SCALE = 1.0 / float(np.sqrt(D))
NEG = -1.0e9

F32 = mybir.dt.float32
BF16 = mybir.dt.bfloat16
NP_BF16 = ml_dtypes.bfloat16

_PROGRAM = None
LAST_RESULT = None


def _build_program():
    nc = bacc.Bacc()
    xTp = nc.declare_dram_parameter("xTp", [128, NSC, NHT, SC], BF16, isOutput=False)
    wq = nc.declare_dram_parameter("wq", [128, NHT, DPH], BF16, isOutput=False)
    wk = nc.declare_dram_parameter("wk", [128, NHT, D], BF16, isOutput=False)
    wv = nc.declare_dram_parameter("wv", [128, NHT, D], BF16, isOutput=False)
    wo = nc.declare_dram_parameter("wo", [128, HPC, HID], BF16, isOutput=False)
    cst = nc.declare_dram_parameter("cst", [128, HPC + 2 + NT + 128], F32, isOutput=False)
    out = nc.declare_dram_parameter("out", [128, NT, HID], BF16, isOutput=True)

    Exp = mybir.ActivationFunctionType.Exp

    with tile.TileContext(nc) as tc:
        with (
            tc.tile_pool(name="consts", bufs=1) as consts,
            tc.tile_pool(name="weights", bufs=1) as wpool,
            tc.tile_pool(name="persist", bufs=1) as persist,
            tc.tile_pool(name="xt", bufs=2) as xtp,
        ):
            ident = consts.tile([128, 128], BF16)
            make_identity(nc, ident[:])
            ones_sq = consts.tile([128, 128], BF16)
            nc.vector.memset(ones_sq[:], 1.0)
            cst_sb = consts.tile([128, HPC + 2 + NT + 128], F32)
            nc.gpsimd.dma_start(cst_sb[:], cst[:])
            bq_sb = cst_sb[:, 0:HPC]
            bkv_sb = cst_sb[:, HPC : HPC + 2]
            padb_sb = cst_sb[:, HPC + 2 : HPC + 2 + NT]
            ltm_sb = cst_sb[:, HPC + 2 + NT :]

            wk_sb = wpool.tile([128, NHT, D], BF16)
            nc.scalar.dma_start(wk_sb[:], wk[:])
            wv_sb = wpool.tile([128, NHT, D], BF16)
            nc.scalar.dma_start(wv_sb[:], wv[:])
            wq_sb = wpool.tile([128, NHT, DPH], BF16)
            wo_sb = wpool.tile([128, HPC, HID], BF16)

            # Persistent activations (live across stages)
            QT = persist.tile([128, HPC, S], BF16)   # Q^T per head: [d, h, q]
            KT = persist.tile([128, S], BF16)        # K^T: [d, kv]
            V = persist.tile([128, NT, 128], BF16)   # V tiles: [kv_p, kv_tile, d]
            OT = persist.tile([128, HPC, S], BF16)   # softmax(S) V, transposed

            with tc.tile_pool(name="vt", bufs=2) as vtp:

                def proj_chunk(sc, pools):
                    """Emit Q/K/V projection work for s-chunk sc.  `pools` is
                    a cycle of (pool, tag) for chain PSUM tiles."""
                    xts = [
                        xtp.tile([128, 4, SC], BF16, tag=f"xt{g}", name=f"xt{g}")
                        for g in range(4)
                    ]
                    for g in range(4):
                        nc.sync.dma_start(
                            xts[g][:], xTp[:, sc, 4 * g : 4 * g + 4]
                        )
                    if sc == 0:
                        nc.sync.dma_start(wq_sb[:], wq[:])

                    def chain(idx, w_sb, dslice):
                        pool, tag = pools[idx % len(pools)]
                        psd = pool.tile([128, SC], F32, tag=tag, name=f"ch{idx}")
                        for ht in range(NHT):
                            nc.tensor.matmul(
                                psd[:], w_sb[:, ht, dslice],
                                xts[ht // 4][:, ht % 4, :],
                                start=(ht == 0), stop=(ht == NHT - 1),
                            )
                        return psd

                    # K^T chunk
                    psk = chain(0, wk_sb, slice(0, D))
                    nc.vector.tensor_scalar_add(
                        KT[:, ts(sc, SC)], psk[:], bkv_sb[:, 0:1]
                    )
                    # V^T chunk -> transpose into V tiles
                    psv = chain(1, wv_sb, slice(0, D))
                    vt_s = vtp.tile([128, SC], BF16, tag="vt")
                    nc.vector.tensor_scalar_add(vt_s[:], psv[:], bkv_sb[:, 1:2])
                    for jj in range(SC // 128):
                        pst = pstr.tile([128, 128], BF16, tag="tr")
                        nc.tensor.transpose(
                            pst[:], vt_s[:, ts(jj, 128)], ident[:]
                        )
                        nc.vector.tensor_copy(V[:, sc * 4 + jj, :], pst[:])
                    # Q^T chunks (4 heads)
                    for dt in range(HPC):
                        psq = chain(2 + dt, wq_sb, slice(dt * 128, dt * 128 + 128))
                        nc.vector.tensor_scalar_add(
                            QT[:, dt, ts(sc, SC)], psq[:], bq_sb[:, dt : dt + 1]
                        )

                def attention(qc2, esp, rsp, psS, psO, psR):
                    # 1024-wide q-chunk: scores for the two 512 halves land in
                    # one flat 2-bank PSUM tile so a single contiguous exp
                    # covers both; fully-masked halves of diagonal k-tiles
                    # are skipped entirely.
                    nkt = 8 * qc2 + 8
                    q0 = qc2 * 2 * SC
                    for h in range(HPC):
                        pso = psO.tile([128, 2 * SC], F32, tag="o", name="pso")
                        # row-sums via ones-matrix matmul: every output
                        # partition gets the same column sum (pre-broadcast)
                        psrb = psR.tile([128, 2 * SC], F32, tag="r", name="psrb")
                        stop0_kt = 8 * qc2 + 3
                        for kt in range(nkt):
                            j = kt - 8 * qc2
                            off = 128 * j if j >= 0 else 0
                            o1 = max(off, SC)
                            ps = psS.tile([128, 2 * SC], F32, tag="s")
                            if off < SC:
                                nc.tensor.matmul(
                                    ps[:, off:SC], KT[:, ts(kt, 128)],
                                    QT[:, h, q0 + off : q0 + SC],
                                    start=True, stop=True,
                                )
                            nc.tensor.matmul(
                                ps[:, o1 : 2 * SC], KT[:, ts(kt, 128)],
                                QT[:, h, q0 + o1 : q0 + 2 * SC],
                                start=True, stop=True,
                            )
                            es = esp.tile([128, 2 * SC], BF16, tag="es")
                            nc.scalar.activation(
                                es[:, off : 2 * SC], ps[:, off : 2 * SC], Exp,
                                bias=padb_sb[:, kt : kt + 1], scale=SCALE,
                            )
                            if j >= 0:
                                # zero the strictly-lower (kv > q) part of the
                                # diagonal 128-block (local cols off:off+128)
                                nc.vector.tensor_mul(
                                    es[:, off : off + 128],
                                    es[:, off : off + 128], ltm_sb
                                )
                            if off < SC:
                                nc.tensor.matmul(
                                    pso[:, off:SC], V[:, kt, :], es[:, off:SC],
                                    start=(kt == 0), stop=(kt == stop0_kt),
                                )
                                nc.tensor.matmul(
                                    psrb[:, off:SC], ones_sq[:], es[:, off:SC],
                                    start=(kt == 0), stop=(kt == stop0_kt),
                                )
                            nc.tensor.matmul(
                                pso[:, o1 : 2 * SC], V[:, kt, :],
                                es[:, o1 : 2 * SC],
                                start=(kt == 0), stop=(kt == nkt - 1),
                            )
                            nc.tensor.matmul(
                                psrb[:, o1 : 2 * SC], ones_sq[:],
                                es[:, o1 : 2 * SC],
                                start=(kt == 0), stop=(kt == nkt - 1),
                            )
                        rs = rsp.tile([128, 2 * SC], F32, tag="rs")
                        nc.vector.tensor_copy(rs[:], psrb[:])
                        bb = rsp.tile([128, 2 * SC], F32, tag="bb")
                        nc.vector.reciprocal_approx_fast(bb[:], rs[:])
                        nc.vector.tensor_mul(
                            OT[:, h, q0 : q0 + 2 * SC], pso[:], bb[:]
                        )

                with (
                    tc.tile_pool(name="p1", bufs=1, space="PSUM") as p1,
                    tc.tile_pool(name="pstr", bufs=2, space="PSUM") as pstr,
                ):
                    pools6 = [(p1, f"c{i}") for i in range(6)]
                    for sc in range(NSC):
                        proj_chunk(sc, pools6)
                with (
                    tc.tile_pool(name="es", bufs=8) as esp,
                    tc.tile_pool(name="rsp", bufs=2) as rsp,
                    tc.tile_pool(name="psS", bufs=2, space="PSUM") as psS,
                    tc.tile_pool(name="psO", bufs=1, space="PSUM") as psO,
                    tc.tile_pool(name="psR", bufs=1, space="PSUM") as psR,
                ):
                    nc.scalar.dma_start(wo_sb[:], wo[:])
                    for qc2 in range(NSC // 2):
                        attention(qc2, esp, rsp, psS, psO, psR)

                # ---------------- output projection ----------------
                with (
                    tc.tile_pool(name="outsb", bufs=3) as outp,
                    tc.tile_pool(name="ps3", bufs=2, space="PSUM") as ps3,
                ):
                    for st in range(NT):
                        ot = outp.tile([128, HID], BF16, tag="ot")
                        for hc in range(HID // SC):
                            pss = ps3.tile([128, SC], F32, tag=f"c3{hc}",
                                           name=f"pss{hc}")
                            for dt in range(HPC):
                                nc.tensor.matmul(
                                    pss[:],
                                    OT[:, dt, ts(st, 128)],
                                    wo_sb[:, dt, ts(hc, SC)],
                                    start=(dt == 0), stop=(dt == HPC - 1),
                                )
                            nc.vector.tensor_copy(ot[:, ts(hc, SC)], pss[:])
                        nc.sync.dma_start(out[:, st, :], ot[:])
    nc.compile()
    return nc


def _get_program():
    global _PROGRAM
    if _PROGRAM is None:
        _PROGRAM = _build_program()
    return _PROGRAM


def _pack_pt(a, p=128):
    """[T*p, N] -> [p, T, N] partition-contiguous."""
    t = a.shape[0] // p
    return np.ascontiguousarray(a.reshape(t, p, *a.shape[1:]).transpose(1, 0, 2))


def kernel(**inputs):
    global LAST_RESULT
    hs = np.ascontiguousarray(inputs["hidden_states"], dtype=np.float32)
    pad = np.ascontiguousarray(inputs["padding_mask"], dtype=np.float32)
    Wq = np.asarray(inputs["Wq"], dtype=np.float32)
    Wk = np.asarray(inputs["Wk"], dtype=np.float32)
    Wv = np.asarray(inputs["Wv"], dtype=np.float32)
    Wo = np.asarray(inputs["Wo"], dtype=np.float32)
    bq_v = np.asarray(inputs["bq"], dtype=np.float32)
    bk_v = np.asarray(inputs["bk"], dtype=np.float32)
    bv_v = np.asarray(inputs["bv"], dtype=np.float32)
    bo_v = np.asarray(inputs["bo"], dtype=np.float32)

    # x[b]^T packed [128, NSC, NHT, SC]: partition p, s-chunk, hid-tile, s'
    xTps = []
    for b in range(B):
        xT = hs[b].T.astype(NP_BF16)           # [HID, S]
        xTps.append(
            np.ascontiguousarray(
                xT.reshape(NHT, 128, NSC, SC).transpose(1, 2, 0, 3)
            )
        )
    WqT = Wq.T  # [HID, HID]
    wk_p = _pack_pt(np.ascontiguousarray(Wk.T).astype(NP_BF16))   # [128,16,128]
    wv_p = _pack_pt(np.ascontiguousarray(Wv.T).astype(NP_BF16))
    WoT = Wo.T  # [HID, HID]

    ltm = np.triu(np.ones((128, 128), np.float32))

    padbs = [(NEG * pad[b]).reshape(NT, 128).T for b in range(B)]
    bqs = [
        bq_v[hg * DPH : (hg + 1) * DPH].reshape(HPC, 128).T for hg in range(HPC)
    ]
    bkv = np.stack([bk_v, bv_v], axis=1)  # [128, 2]
    csts = {}
    for b in range(B):
        for hg in range(HPC):
            csts[(b, hg)] = np.ascontiguousarray(
                np.concatenate([bqs[hg], bkv, padbs[b], ltm], axis=1)
            ).astype(np.float32)

    wq_ps = [
        _pack_pt(
            np.ascontiguousarray(WqT[:, hg * DPH : (hg + 1) * DPH]).astype(NP_BF16)
        )
        for hg in range(HPC)
    ]
    wo_ps = [
        _pack_pt(
            np.ascontiguousarray(WoT[hg * DPH : (hg + 1) * DPH, :]).astype(NP_BF16)
        )
        for hg in range(HPC)
    ]

    nc = _get_program()
    in_maps = []
    for c in range(NCORES):
        b, hg = c // 4, c % 4
        in_maps.append(
            {
                "xTp": xTps[b],
                "wq": wq_ps[hg],
                "wk": wk_p,
                "wv": wv_p,
                "wo": wo_ps[hg],
                "cst": csts[(b, hg)],
            }
        )

    LAST_RESULT = run_bass_kernel_spmd(nc, in_maps, list(range(NCORES)))
    res = LAST_RESULT.results

    outp = np.zeros((B, S, HID), np.float32)
    for c in range(NCORES):
        part = np.asarray(res[c]["out"], dtype=np.float32)  # [128, NT, HID]
        outp[c // 4] += part.transpose(1, 0, 2).reshape(S, HID)
    outp += bo_v[None, None, :]
    return outp


if __name__ == "__main__":
    rng = np.random.default_rng(0)
    demo = {
        "hidden_states": rng.standard_normal((B, S, HID), dtype=np.float32),
        "causal_mask": np.triu(np.ones((1, 1, S, S), np.float32), k=1),
        "padding_mask": np.zeros((B, S), np.float32),
        "Wq": (rng.standard_normal((HID, HID), dtype=np.float32) * 0.02),
        "bq": np.zeros((HID,), np.float32),
        "Wk": (rng.standard_normal((D, HID), dtype=np.float32) * 0.02),
        "bk": np.zeros((D,), np.float32),
        "Wv": (rng.standard_normal((D, HID), dtype=np.float32) * 0.02),
        "bv": np.zeros((D,), np.float32),
        "Wo": (rng.standard_normal((HID, HID), dtype=np.float32) * 0.02),
        "bo": np.zeros((HID,), np.float32),
    }
    o = kernel(**demo)
    print("kernel output", o.shape, o.dtype, float(np.abs(o).mean()))
